# revision 12
# baseline (speedup 1.0000x reference)
"""Trainium2 Bass kernel for the channel-interaction-attention module.

Reference computation (x: (4, 1024, 64, 64) fp32, F = x.ravel()):
    A  = F.view(16384, 1024)          # x.reshape(-1, C)
    Bm = F.view(1024, 16384)          # x.reshape(C, -1)
    S  = Bm @ A                       # (C, C)
    E  = softmax(S, axis=-1)
    U  = E @ Bm                       # (C, N)
    Y  = softmax(U, axis=-1)          # softmax over N = 16384
    out = x + softmax(Y.view(4,1024,64,64), axis=-1)   # softmax over W=64

Sharding: N = 16384 split into 8 column-shards of 2048 (one per core).
GEMM1 contracts over the shard -> per-core partial S, summed with an
AllReduce (bf16, split into two halves so the collective overlaps GEMM
work); softmax(S) is replicated.  GEMM2 + the N-softmax row-sums use one
tiny (4 KiB) AllReduce.  The W-softmax and residual are shard-local.

GEMMs run in fp8-e4m3 DoubleRow mode (2 MACs/cell/cycle).  The triple
softmax makes this numerically safe: S's row-softmax is a near-hard max
(logit std ~128), so GEMM noise only perturbs the tiny non-argmax attn
weights, and the final output is dominated by the fp32 residual x.
"""

import numpy as np
import ml_dtypes

import concourse.bass as bass
import concourse.bacc as bacc
import concourse.tile as tile
import concourse.mybir as mybir
from concourse import bass_utils

N_CORES = 8
B, C, H, W = 4, 1024, 64, 64
N = B * H * W            # 16384
NS = N // N_CORES        # 2048 per-core shard
P = 128
MT = C // P              # 8 row tiles of S / U
KT1 = NS // P            # 16 x 128 contraction tiles for GEMM1
KT2 = C // P             # 8 x 128 contraction tiles for GEMM2
NCH1 = C // 512          # 2 n-chunks (512) for GEMM1
NCH2 = NS // 512         # 4 n-chunks (512) for GEMM2
HALVES = 2
MH = MT // HALVES        # m-tiles per half

FP32 = mybir.dt.float32
BF16 = mybir.dt.bfloat16
FP8 = mybir.dt.float8e4
EXP = mybir.ActivationFunctionType.Exp
DR = mybir.MatmulPerfMode.DoubleRow


def build_module(repeat: int = 1, fp8: bool = True, collectives: bool = True):
    nc = bacc.Bacc("TRN2", target_bir_lowering=False, debug=False,
                   num_devices=N_CORES if collectives else 1)

    def all_reduce(cc_in, cc_out):
        if collectives:
            nc.gpsimd.collective_compute(
                "AllReduce", mybir.AluOpType.add,
                replica_groups=[list(range(N_CORES))],
                ins=[cc_in.opt()], outs=[cc_out.opt()],
            )
        else:
            nc.sync.dma_start(cc_out[:], cc_in[:])

    IN_DT = FP8 if fp8 else BF16
    a_d = nc.dram_tensor("a_in", [NS, C], IN_DT, kind="ExternalInput")
    bt_d = nc.dram_tensor("bt_in", [NS, C], IN_DT, kind="ExternalInput")
    b_d = nc.dram_tensor("b_in", [C, NS], IN_DT, kind="ExternalInput")
    bf_d = nc.dram_tensor("bf_in", [C, NS], FP32, kind="ExternalInput")
    id_d = nc.dram_tensor("id_in", [P, P], BF16, kind="ExternalInput")
    o_d = nc.dram_tensor("o_out", [C, NS], FP32, kind="ExternalOutput")

    def mm1(ps, m, nn, kk):
        """GEMM1 matmul for contraction tile kk (of KT1)."""
        if fp8:
            nc.tensor.matmul(
                ps[:],
                bt_t[:, 2 * kk:2 * kk + 2, m * P:(m + 1) * P],
                a_t[:, 2 * kk:2 * kk + 2, nn * 512:(nn + 1) * 512],
                start=(kk == 0), stop=(kk == KT1 // 2 - 1), perf_mode=DR)
        else:
            nc.tensor.matmul(
                ps[:],
                bt_t[:, kk, m * P:(m + 1) * P],
                a_t[:, kk, nn * 512:(nn + 1) * 512],
                start=(kk == 0), stop=(kk == KT1 - 1))

    def mm2(ps, m, nn, kk):
        if fp8:
            nc.tensor.matmul(
                ps[:],
                et_t[:, 2 * kk:2 * kk + 2, m * P:(m + 1) * P],
                b_t[:, 2 * kk:2 * kk + 2, nn * 512:(nn + 1) * 512],
                start=(kk == 0), stop=(kk == KT2 // 2 - 1), perf_mode=DR)
        else:
            nc.tensor.matmul(
                ps[:],
                et_t[:, kk, m * P:(m + 1) * P],
                b_t[:, kk, nn * 512:(nn + 1) * 512],
                start=(kk == 0), stop=(kk == KT2 - 1))

    K1 = KT1 // 2 if fp8 else KT1
    K2 = KT2 // 2 if fp8 else KT2

    with tile.TileContext(nc) as tc:
        with (
            tc.tile_pool(name="big", bufs=1) as big,
            tc.tile_pool(name="epool", bufs=1) as epool,
            tc.tile_pool(name="schunk", bufs=2) as schunk,
            tc.tile_pool(name="srchunk", bufs=1) as srchunk,
            tc.tile_pool(name="stat", bufs=1) as stat,
            tc.tile_pool(name="zp", bufs=2) as zp,
            tc.tile_pool(name="ps1", bufs=4, space="PSUM") as ps1,
            tc.tile_pool(name="pst", bufs=4, space="PSUM") as pst,
            tc.tile_pool(name="dram", bufs=1, space="DRAM") as dram,
        ):
            ident = stat.tile([P, P], BF16, tag="ident")
            nc.sync.dma_start(ident[:], id_d[:])

            for rep in range(repeat):
                # ---- load GEMM1/GEMM2 operands (batched strided DMAs) ----
                a_t = big.tile([P, KT1, C], IN_DT, tag="a")
                bt_t = big.tile([P, KT1, C], IN_DT, tag="bt")
                a_r = a_d[:].rearrange("(kk p) m -> p kk m", p=P)
                bt_r = bt_d[:].rearrange("(kk p) m -> p kk m", p=P)
                KB = 4  # k-tiles per DMA
                for kk in range(0, KT1, KB):
                    nc.sync.dma_start(a_t[:, kk:kk + KB, :], a_r[:, kk:kk + KB, :])
                    nc.sync.dma_start(bt_t[:, kk:kk + KB, :],
                                      bt_r[:, kk:kk + KB, :])
                b_t = big.tile([P, KT2, NS], IN_DT, tag="b")
                b_r = b_d[:].rearrange("(kk p) n -> p kk n", p=P)
                for kk in range(0, KT2, KB):
                    nc.sync.dma_start(b_t[:, kk:kk + KB, :], b_r[:, kk:kk + KB, :])
                # residual rows, loaded via the software-DGE path (gpsimd)
                bf_t = big.tile([P, MT, NS], FP32, tag="bf")
                bf_r = bf_d[:].rearrange("(mm p) n -> p mm n", p=P)
                for mm in range(0, MT, 2):
                    nc.gpsimd.dma_start(bf_t[:, mm:mm + 2, :],
                                        bf_r[:, mm:mm + 2, :])

                # ---- GEMM1 (by halves): partial S_k = Bm_k @ A_k, bf16 ----
                s_cc_in = [dram.tile([P, MH, C], BF16, tag=f"ccin{h}",
                                     name=f"s_cc_in{h}_{rep}")
                           for h in range(HALVES)]
                s_cc_out = [dram.tile([P, MH, C], BF16, tag=f"ccout{h}",
                                      addr_space="Shared",
                                      name=f"s_cc_out{h}_{rep}")
                            for h in range(HALVES)]
                for h in range(HALVES):
                    sh = schunk.tile([P, MH, C], BF16, tag="sh",
                                     name=f"sh_{rep}_{h}")
                    for mm in range(MH):
                        m = h * MH + mm
                        for nn in range(NCH1):
                            ps = ps1.tile([P, 512], FP32, tag="ps",
                                          name=f"ps_{rep}_{m}_{nn}")
                            for kk in range(K1):
                                mm1(ps, m, nn, kk)
                            nc.vector.tensor_copy(
                                sh[:, mm, nn * 512:(nn + 1) * 512], ps[:])
                    nc.sync.dma_start(s_cc_in[h][:], sh[:])
                    all_reduce(s_cc_in[h], s_cc_out[h])

                # ---- per half: softmax(S) -> E, transpose, GEMM2 ----
                negmax = stat.tile([P, MT], FP32, tag="negmax")
                rsum = stat.tile([P, MT], FP32, tag="rsum")
                rscale = stat.tile([P, MT], FP32, tag="rscale")
                e_t = epool.tile([P, MT, C], BF16, tag="e")
                et_t = epool.tile([P, KT2, C], IN_DT, tag="et")
                u_t = big.tile([P, MT, NS], BF16, tag="u")
                acc4 = stat.tile([P, MT, NCH2], FP32, tag="acc4")
                for h in range(HALVES):
                    srh = srchunk.tile([P, MH, C], BF16, tag="sr",
                                       name=f"sr_{rep}_{h}")
                    nc.sync.dma_start(srh[:], s_cc_out[h][:])
                    for mm in range(MH):
                        m = h * MH + mm
                        sr = srh[:, mm, :]
                        nc.vector.tensor_reduce(
                            negmax[:, m:m + 1], sr[:],
                            axis=mybir.AxisListType.X, op=mybir.AluOpType.max,
                            negate=True)
                        nc.scalar.activation(
                            e_t[:, m, :], sr[:], EXP,
                            bias=negmax[:, m:m + 1], scale=1.0,
                            accum_out=rsum[:, m:m + 1])
                        nc.vector.reciprocal(rscale[:, m:m + 1],
                                             rsum[:, m:m + 1])
                        for j in range(MT):
                            pt = pst.tile([P, P], BF16, tag="pt",
                                          name=f"pt_{rep}_{m}_{j}")
                            nc.tensor.transpose(
                                pt[:], e_t[:, m, j * P:(j + 1) * P], ident[:])
                            nc.vector.tensor_copy(
                                et_t[:, j, m * P:(m + 1) * P], pt[:])
                    # GEMM2 for this half's output rows
                    for mm in range(MH):
                        m = h * MH + mm
                        for nn in range(NCH2):
                            ps = ps1.tile([P, 512], FP32, tag="ps",
                                          name=f"ps2_{rep}_{m}_{nn}")
                            for kk in range(K2):
                                mm2(ps, m, nn, kk)
                            nc.scalar.activation(
                                u_t[:, m, nn * 512:(nn + 1) * 512], ps[:], EXP,
                                bias=0.0, scale=rscale[:, m:m + 1],
                                accum_out=acc4[:, m, nn:nn + 1])

                # local row sums of exp(U) -> AllReduce -> 1/gsum
                ls_in = dram.tile([P, MT], FP32, tag="lsin")
                lsum = stat.tile([P, MT], FP32, tag="lsum")
                nc.vector.tensor_reduce(lsum[:], acc4[:],
                                        axis=mybir.AxisListType.X,
                                        op=mybir.AluOpType.add)
                nc.sync.dma_start(ls_in[:], lsum[:])
                ls_out = dram.tile([P, MT], FP32, tag="lsout",
                                   addr_space="Shared")
                all_reduce(ls_in, ls_out)
                gsum = stat.tile([P, MT], FP32, tag="gsum")
                nc.sync.dma_start(gsum[:], ls_out[:])
                gscale = stat.tile([P, MT], FP32, tag="gscale")
                nc.vector.reciprocal(gscale[:], gsum[:])

                # ---- per-m: z = exp(u * gscale); W-softmax; + residual ----
                o_r = o_d[:].rearrange("(mm p) n -> p mm n", p=P)
                for m in range(MT):
                    bfm = bf_t[:, m, :]
                    z = zp.tile([P, NS], BF16, tag="z", name=f"z_{rep}_{m}")
                    nc.scalar.activation(z[:], u_t[:, m, :], EXP,
                                         bias=0.0, scale=gscale[:, m:m + 1])
                    z3 = z[:].rearrange("p (r w) -> p r w", w=W)
                    wsum = stat.tile([P, NS // W], FP32, tag="wsum",
                                     name=f"wsum_{rep}_{m}")
                    nc.vector.tensor_reduce(wsum[:], z3,
                                            axis=mybir.AxisListType.X,
                                            op=mybir.AluOpType.add)
                    wrecip = stat.tile([P, NS // W], FP32, tag="wrecip",
                                       name=f"wrecip_{rep}_{m}")
                    nc.vector.reciprocal(wrecip[:], wsum[:])
                    wb = wrecip[:].unsqueeze(2).broadcast_to((P, NS // W, W))
                    # alternate the two big elementwise passes DVE/GpSimd
                    eng = nc.vector if (m % 2 == 0) else nc.gpsimd
                    eng.tensor_tensor(z3, z3, wb, op=mybir.AluOpType.mult)
                    eng.tensor_tensor(bfm[:], bfm[:], z[:],
                                      op=mybir.AluOpType.add)
                    if m % MH == MH - 1:
                        h = m // MH
                        nc.sync.dma_start(
                            o_r[:, h * MH:(h + 1) * MH, :],
                            bf_t[:, h * MH:(h + 1) * MH, :])

    nc.compile()
    return nc


_module_cache = {}


def _get_module(repeat: int = 1, fp8: bool = True, collectives: bool = True):
    key = (repeat, fp8, collectives)
    if key not in _module_cache:
        _module_cache[key] = build_module(repeat, fp8, collectives)
    return _module_cache[key]


def make_in_maps(x: np.ndarray, fp8: bool = True):
    in_dt = ml_dtypes.float8_e4m3 if fp8 else ml_dtypes.bfloat16
    F = np.ascontiguousarray(x, dtype=np.float32).reshape(-1)
    A = F.reshape(N, C)
    Bm = F.reshape(C, N)
    ident = np.eye(P, dtype=ml_dtypes.bfloat16)
    in_maps = []
    for k in range(N_CORES):
        sl = slice(k * NS, (k + 1) * NS)
        b_f32 = np.ascontiguousarray(Bm[:, sl])
        b_lp = b_f32.astype(in_dt)
        bt_lp = np.ascontiguousarray(b_lp.T)
        a_lp = A[sl].astype(in_dt)
        in_maps.append({
            "a_in": a_lp,
            "bt_in": bt_lp,
            "b_in": b_lp,
            "bf_in": b_f32,
            "id_in": ident,
        })
    return in_maps


def assemble_output(x: np.ndarray, results):
    out = np.concatenate([results[k]["o_out"] for k in range(N_CORES)], axis=1)
    return out.reshape(B, C, H, W).astype(np.float32)


def kernel(x: np.ndarray) -> np.ndarray:
    nc = _get_module()
    in_maps = make_in_maps(x)
    res = bass_utils.run_bass_kernel_spmd(
        nc, in_maps, core_ids=list(range(N_CORES)))
    return assemble_output(x, res.results)
